# revision 1
# baseline (speedup 1.0000x reference)
"""GPTQ int4 dequant + matmul kernel for Trainium2, column-parallel over 8 cores.

Computes out = x @ dequant(qweight, qzeros, scales) + bias where
  qweight: [OC//8, IC_total] int32 (nibbles packed along OC rows)
  qzeros:  [G, IC_total//8]  int32 (nibbles packed along IC cols)
  scales:  [G, IC_total]     float32
  x:       [N, OC]           float32
  bias:    [IC_total]        float32
Sharding: IC (out_features) split across 8 cores; x replicated.

Per-core kernel structure:
  1. zp unpack (strided shift/mask) + bias-bit trick (|0x4B000000 so the int
     nibble bits are exactly the fp32 value 2^23+zp) -> PE-transpose to
     [IC, G] layout so zp/s become per-partition scalars.
  2. qweight: DMA -> PE-transpose (int32, bit-exact permutation) to
     [IC, OC//8] layout; unpack nibbles with immediate shifts (strided
     free-dim writes); OR 0x4B000000; one fused tensor_scalar per group:
     W^T = ((2^23+nib) - (2^23+zp)) * s  -> bf16.  All bit-exact int ops +
     exact float ops; single rounding to bf16.
  3. dma_start_transpose W^T -> W [OC part, IC free] (bf16, xbar).
  4. Main loop over 128-row token tiles: gpsimd cast-DMA x (fp32->bf16),
     dma_start_transpose -> xT tiles; matmul with xT stationary, W streamed
     from SBUF, fp32 psum accumulation over OC; bias added via a K=1 matmul
     with a ones row; ACT drains psum -> SBUF; DMA out.
"""

import sys

if "/opt/trn_rl_repo" not in sys.path:
    sys.path.insert(0, "/opt/trn_rl_repo")

from contextlib import ExitStack

import numpy as np
import ml_dtypes

from concourse import bacc, bass, mybir, tile

P = 128
PACK = 8
FP32_BIAS_BITS = 0x4B000000  # fp32 bit pattern of 2**23
FP32_BIAS = float(2**23)

f32 = mybir.dt.float32
bf16 = mybir.dt.bfloat16
i32 = mybir.dt.int32
Alu = mybir.AluOpType

# Full problem dims (hardcoded per harness contract)
N_FULL = 4096
K_FULL = 4096  # OC / in_features (contraction)
IC_TOTAL = 11008
G_FULL = 32
N_CORES = 8
IC_SHARD = IC_TOTAL // N_CORES  # 1376


def _jtiles(ic):
    """IC j-tiles of <=128, last may be ragged (must stay %16 for xbar)."""
    tiles = []
    off = 0
    while off < ic:
        w = min(P, ic - off)
        assert w % 16 == 0, f"ragged j-tile {w} not multiple of 16"
        tiles.append((off, w))
        off += ic and w
    return tiles


def _chunks(ic):
    """Greedy grouping of j-tiles into psum chunks of <=512 fp32."""
    chunks = []
    start = 0
    for off, w in _jtiles(ic):
        if off + w - start > 512:
            chunks.append((start, off - start))
            start = off
    chunks.append((start, ic - start))
    return chunks


def build(nc, n=N_FULL, k=K_FULL, ic=IC_SHARD, g=G_FULL):
    """Emit the per-core program. All cores run the same program (SPMD)."""
    assert k % P == 0 and n % P == 0 and k // g == P
    KT = k // P  # contraction tiles (each == one quant group)
    NT = n // P  # token tiles
    jts = _jtiles(ic)
    chunks = _chunks(ic)
    # map j-tile -> (chunk index, offset within chunk)
    jt_chunk = []
    for off, w in jts:
        for ci, (c0, cw) in enumerate(chunks):
            if c0 <= off < c0 + cw:
                jt_chunk.append((ci, off - c0))
                break

    q_d = nc.dram_tensor("qweight", [k // PACK, ic], i32, kind="ExternalInput")
    qz_d = nc.dram_tensor("qzeros", [g, ic // PACK], i32, kind="ExternalInput")
    s_d = nc.dram_tensor("scales", [g, ic], f32, kind="ExternalInput")
    x_d = nc.dram_tensor("x", [n, k], f32, kind="ExternalInput")
    b_d = nc.dram_tensor("bias", [ic], f32, kind="ExternalInput")
    id128_d = nc.dram_tensor("id128_f32", [P, P], f32, kind="ExternalInput")
    idg_f_d = nc.dram_tensor("idg_f32", [g, g], f32, kind="ExternalInput")
    ones_d = nc.dram_tensor("ones_row", [1, P], bf16, kind="ExternalInput")
    out_d = nc.dram_tensor("out", [n, ic], f32, kind="ExternalOutput")

    with tile.TileContext(nc) as tc, ExitStack() as ctx:
        const = ctx.enter_context(tc.tile_pool(name="const", bufs=1))
        wpool = ctx.enter_context(tc.tile_pool(name="w", bufs=1))
        prep = ctx.enter_context(tc.tile_pool(name="prep", bufs=2))
        prep1 = ctx.enter_context(tc.tile_pool(name="prep1", bufs=1))
        xpool = ctx.enter_context(tc.tile_pool(name="x", bufs=2))
        opool = ctx.enter_context(tc.tile_pool(name="o", bufs=2))
        psum = ctx.enter_context(tc.tile_pool(name="psum", bufs=2, space="PSUM"))
        psum_t = ctx.enter_context(tc.tile_pool(name="psum_t", bufs=2, space="PSUM"))

        # ---- constants
        id128 = const.tile([P, P], f32)
        nc.sync.dma_start(out=id128[:], in_=id128_d[:])
        idg_f = const.tile([g, g], f32)
        nc.sync.dma_start(out=idg_f[:], in_=idg_f_d[:])
        ones = const.tile([1, P], bf16)
        nc.sync.dma_start(out=ones[:], in_=ones_d[:])
        bias_row = const.tile([1, ic], bf16)
        nc.gpsimd.dma_start(out=bias_row[:], in_=b_d[None, :])  # cast f32->bf16

        # ---- zp unpack: qzeros [g, ic//8] -> zp_or [g, ic] (bits = fp32 2^23+zp)
        qz_sb = const.tile([g, ic // PACK], i32)
        nc.sync.dma_start(out=qz_sb[:], in_=qz_d[:])
        zp_or = const.tile([g, ic], i32)
        for r in range(PACK):
            nc.vector.tensor_scalar(
                out=zp_or[:, r::PACK],
                in0=qz_sb[:],
                scalar1=4 * r,
                scalar2=15,
                op0=Alu.logical_shift_right,
                op1=Alu.bitwise_and,
            )
        nc.vector.tensor_scalar(
            out=zp_or[:], in0=zp_or[:], scalar1=FP32_BIAS_BITS, scalar2=None,
            op0=Alu.bitwise_or,
        )
        s_sb = const.tile([g, ic], f32)
        nc.sync.dma_start(out=s_sb[:], in_=s_d[:])

        # ---- transpose zp_or and scales to [IC-part, g] layout
        NJ = len(jts)
        zpT = const.tile([P, NJ, g], f32)  # bits are fp32 2^23+zp already
        sT = const.tile([P, NJ, g], f32)
        for ji, (off, w) in enumerate(jts):
            pz = psum_t.tile([P, P], f32, name="pst_f")
            nc.tensor.transpose(
                pz[:w, :g], zp_or.bitcast(f32)[:, off : off + w], idg_f[:]
            )
            nc.vector.tensor_copy(zpT[:w, ji, :], pz[:w, :g])
            ps_ = psum_t.tile([P, P], f32, name="pst_f")
            nc.tensor.transpose(ps_[:w, :g], s_sb[:, off : off + w], idg_f[:])
            nc.vector.tensor_copy(sT[:w, ji, :], ps_[:w, :g])

        # ---- W chunks in [OC-part, KT, chunk-width] bf16
        wtiles = [wpool.tile([P, KT, cw], bf16, name=f"Wc{ci}")
                  for ci, (c0, cw) in enumerate(chunks)]

        RP = k // PACK  # packed qweight rows
        rts = [(r0, min(P, RP - r0)) for r0 in range(0, RP, P)]
        for ji, (off, w) in enumerate(jts):
            # load qweight columns [off:off+w] as [<=128, n_rt, w]
            qw4 = prep.tile([P, len(rts), P], i32, name="qw4")
            for rt, (r0, rw) in enumerate(rts):
                nc.sync.dma_start(
                    out=qw4[:rw, rt, :w],
                    in_=q_d[r0 : r0 + rw, off : off + w],
                )
            # PE-transpose (bit-exact) -> qwT [w, k//8 packed rows]
            qwT = prep.tile([P, RP], i32, name="qwT")
            for rt, (r0, rw) in enumerate(rts):
                pq = psum_t.tile([P, P], f32, name="pst_f")
                nc.tensor.transpose(
                    pq[:w, :rw], qw4.bitcast(f32)[:rw, rt, :w], id128[:rw, :rw]
                )
                nc.vector.tensor_copy(qwT.bitcast(f32)[:w, r0 : r0 + rw], pq[:w, :rw])
            qwT_flat = qwT[:w, :]

            # unpack nibbles: nib[j, 8r+kk] = (qwT[j, r] >> 4kk) & 15
            nib = prep.tile([P, k], i32, name="nib")
            for kk in range(PACK):
                nc.vector.tensor_scalar(
                    out=nib[:w, kk::PACK],
                    in0=qwT_flat,
                    scalar1=4 * kk,
                    scalar2=15,
                    op0=Alu.logical_shift_right,
                    op1=Alu.bitwise_and,
                )
            nc.vector.tensor_scalar(
                out=nib[:w, :], in0=nib[:w, :], scalar1=FP32_BIAS_BITS,
                scalar2=None, op0=Alu.bitwise_or,
            )
            # dequant: WT = ((2^23+nib) - (2^23+zp)) * s -> bf16
            wt = prep.tile([P, k], bf16, name="wt")
            nibf = nib.bitcast(f32)
            for gi in range(g):
                nc.vector.tensor_scalar(
                    out=wt[:w, gi * P : (gi + 1) * P],
                    in0=nibf[:w, gi * P : (gi + 1) * P],
                    scalar1=zpT[:w, ji, gi : gi + 1],
                    scalar2=sT[:w, ji, gi : gi + 1],
                    op0=Alu.subtract,
                    op1=Alu.mult,
                )
            # xbar transpose WT [w, k] -> W [OC-part, KT, j-slice]
            ci, coff = jt_chunk[ji]
            nc.sync.dma_start_transpose(
                out=wtiles[ci][:, :, coff : coff + w], in_=wt[:w, :]
            )

        # ---- main loop over token tiles
        for nt in range(NT):
            xb = xpool.tile([P, k], bf16, name="xb")
            nc.gpsimd.dma_start(out=xb[:], in_=x_d[nt * P : (nt + 1) * P, :])
            xT = xpool.tile([P, KT, P], bf16, name="xT")
            nc.sync.dma_start_transpose(out=xT[:], in_=xb[:])

            ps = psum.tile([P, ic], f32, name="ps")
            for kt in range(KT):
                for ci, (c0, cw) in enumerate(chunks):
                    nc.tensor.matmul(
                        ps[:, c0 : c0 + cw],
                        lhsT=xT[:, kt, :],
                        rhs=wtiles[ci][:, kt, :],
                        start=(kt == 0),
                        stop=False,
                    )
            # bias via K=1 matmul with ones row (also closes the accum group)
            for ci, (c0, cw) in enumerate(chunks):
                nc.tensor.matmul(
                    ps[:, c0 : c0 + cw],
                    lhsT=ones[:, :],
                    rhs=bias_row[:, c0 : c0 + cw],
                    start=False,
                    stop=True,
                )
            out_sb = opool.tile([P, ic], f32, name="out_sb")
            nc.scalar.copy(out=out_sb[:], in_=ps[:])
            nc.sync.dma_start(
                out=out_d[nt * P : (nt + 1) * P, :], in_=out_sb[:]
            )
    return nc


def make_const_inputs(g=G_FULL):
    return {
        "id128_f32": np.eye(P, dtype=np.float32),
        "idg_f32": np.eye(g, dtype=np.float32),
        "ones_row": np.ones((1, P), dtype=ml_dtypes.bfloat16),
    }


def kernel(input, qweight, qzeros, scales, bias):
    """Full-problem entry point: shard, run on 8 cores, gather."""
    from concourse.bass_utils import run_bass_kernel_spmd

    nc = bacc.Bacc("TRN2", target_bir_lowering=False, debug=False)
    build(nc)
    nc.compile()

    consts = make_const_inputs()
    x = np.ascontiguousarray(input, dtype=np.float32)
    in_maps = []
    for c in range(N_CORES):
        j0, j1 = c * IC_SHARD, (c + 1) * IC_SHARD
        in_maps.append(
            {
                "qweight": np.ascontiguousarray(qweight[:, j0:j1]),
                "qzeros": np.ascontiguousarray(
                    qzeros[:, c * (IC_SHARD // PACK) : (c + 1) * (IC_SHARD // PACK)]
                ),
                "scales": np.ascontiguousarray(scales[:, j0:j1]),
                "x": x,
                "bias": np.ascontiguousarray(bias[j0:j1]),
                **consts,
            }
        )
    res = run_bass_kernel_spmd(nc, in_maps, list(range(N_CORES)))
    outs = [np.asarray(res.results[c]["out"], dtype=np.float32) for c in range(N_CORES)]
    return np.concatenate(outs, axis=1)

